# revision 20
# baseline (speedup 1.0000x reference)
"""MemNN (end-to-end memory network) Trainium2 kernel.

Algorithmic restructuring vs the naive reference: the reference materializes
A_h = facts @ Wa[h] (B,L,D) only to contract it with u into match = A_h . u.
Reassociating, match[b,l] = facts[b,l,:] . (Wa[h]^T u[b]) — the (B,L,D)
A-tensor never needs to exist, killing half of the 98.3 GFLOP.  What remains
u-independent (and therefore device-batchable in one launch) is the output
embedding C_h = facts @ Wc[h]: the three hops fuse into a single
(3200, 10000) @ (10000, 768) matmul = 49.2 GFLOP, which this kernel runs in
bf16 across 8 cores.  The sequential hop recurrence (match via the
reassociated form, softmax, att = p.C, Dense update) is ~1.3 GFLOP and runs
on the host, where `facts` is already resident — a device round-trip per hop
would cost ~100 ms of RPC latency for ~15 us of device math.

Sharding: vocab (contraction) dim split 8 ways -> each core reads only its
1/8 slice of facts/Wc (~10 MB/core), computes a partial product at full PE
rate (bf16), and writes it to DRAM.  The host unshards by summing the 8
partials in fp32.

Precision: bf16 inputs with fp32 PSUM accumulation; empirically the
end-to-end relative error is ~6e-3 against the fp32 reference (gate 2e-2).
"""

import os

os.environ.setdefault("MYCRO_LOCAL_CACHE", "1")

import numpy as np

import concourse.bass as bass
import concourse.mybir as mybir
import concourse.tile as tile
from concourse.bass_utils import run_bass_kernel_spmd

HOPS, B, L, V, D = 3, 64, 50, 10000, 256
NCORES = 8
BL = B * L                # 3200 moving rows
NF = HOPS * D             # 768 fused output cols: [Wc0|Wc1|Wc2]
VSH = V // NCORES         # 1250 vocab rows per core
KT = 10                   # contraction tiles of 128 per core
VPAD = KT * 128           # 1280 (zero-padded)
MCH = 512                 # moving-col chunk (512 = full PSUM bank; fewest matmuls)
NN = NF // 128            # 6 stationary W tiles
BF16 = mybir.dt.bfloat16
F32 = mybir.dt.float32

_nc_cache = None
_last_result = None       # BassKernelResults of the most recent run (for profiling)


def _legalize_sync(nc):
    """Split multi-wait sync_info into standalone single-wait EventSemaphores.

    The walrus build in this environment enforces the raw-bass contract of at
    most ONE SyncWait per instruction ("Too many sync wait commands" in
    setupSyncWait otherwise), while Tile attaches every needed wait to the
    consuming instruction.  Hoisting all-but-one wait onto preceding
    InstEventSemaphore instructions on the same engine queue is semantically
    identical: engine queues are in-order, so a preceding wait blocks the
    queue exactly like an attached wait.  Updates are left untouched (they
    fire at completion and cannot be hoisted).
    """
    for func in nc.m.functions:
        for block in func.blocks:
            insts = list(block.instructions)
            out = []
            n = 0
            for inst in insts:
                si = inst.sync_info
                if si is not None and len(si.on_wait) > 1:
                    waits = list(si.on_wait)
                    for w in waits[:-1]:
                        ev = mybir.InstEventSemaphore(
                            name=f"{inst.name}-hoistw{n}", ins=[], outs=[]
                        )
                        n += 1
                        ev.engine = inst.engine
                        ev.sync_info = mybir.SyncInfo(on_wait=[w], on_update=[])
                        nc.register_instruction(ev)
                        out.append(ev)
                    inst.sync_info = mybir.SyncInfo(
                        on_wait=[waits[-1]], on_update=list(si.on_update)
                    )
                out.append(inst)
            if len(out) != len(insts):
                block.instructions = out
    return nc


# Moving-dim chunking of the 3200 BL columns.
def _chunk_widths(mch):
    if mch == 512:
        return [512] * 6 + [128]      # 3200 = 6*512 + 128
    assert BL % mch == 0, mch
    return [mch] * (BL // mch)


_WIDTHS = _chunk_widths(MCH)
_STARTS = [sum(_WIDTHS[:i]) for i in range(len(_WIDTHS))]
assert sum(_WIDTHS) == BL


def _build(reps=1, redma=True, dma_out=True, copy_out=True, xbufs=3, mch=None,
           mwidth=None, hw_loop=False):
    """Build the SPMD device program.

    reps>1 repeats the main loop body (same data, same output addresses) —
    used only by the benchmark harness to measure device time differentially
    (per-call dispatch noise over the axon tunnel is ~ms, device time is
    ~200 us, so wall-clocking one launch cannot resolve it).

    redma/dma_out/copy_out are micro-benchmark toggles (timing variants
    only): re-DMA facts each rep / write pc_t / evacuate PSUM.
    """
    widths = _chunk_widths(mch) if mch else _WIDTHS
    starts = [sum(widths[:i]) for i in range(len(widths))]
    nc = bass.Bass(trn_type="TRN2")
    facts_t = nc.dram_tensor("facts_t", [VPAD, BL], BF16, kind="ExternalInput")
    wac = nc.dram_tensor("wac", [VPAD, NF], BF16, kind="ExternalInput")
    pc_t = nc.dram_tensor("pc_t", [NF, BL], BF16, kind="ExternalOutput")

    fr = facts_t.rearrange("(k p) n -> p k n", p=128)
    wr = wac.rearrange("(k p) n -> p k n", p=128)
    wmax = max(widths)

    with (
        tile.TileContext(nc) as tc,
        tc.tile_pool(name="wpool", bufs=1) as wpool,
        tc.tile_pool(name="xpool", bufs=xbufs) as xpool,
        tc.tile_pool(name="opool", bufs=4) as opool,
        tc.tile_pool(name="pspool", bufs=6, space="PSUM") as pspool,
    ):
        # Prologue DMA order: first wac n-slice 0 + first facts chunk (the
        # first matmul group's deps), then the rest of wac.
        wt = wpool.tile([128, KT, NF], BF16)
        nc.sync.dma_start(wt[:, :, 0:128], wr[:, :, 0:128])
        xts = {}
        xts[0] = xpool.tile(
            [128, KT, widths[0]], BF16, tag="xt", name="xt",
            padded_shape=[128, KT, wmax],
        )
        nc.sync.dma_start(xts[0][:], fr[:, :, 0 : widths[0]])
        for off in range(128, NF, 512):
            end = min(off + 512, NF)
            nc.sync.dma_start(wt[:, :, off:end], wr[:, :, off:end])

        def get_xt(mi):
            if mi not in xts:
                xts[mi] = xpool.tile(
                    [128, KT, widths[mi]], BF16, tag="xt", name="xt",
                    padded_shape=[128, KT, wmax],
                )
                nc.sync.dma_start(
                    xts[mi][:], fr[:, :, starts[mi] : starts[mi] + widths[mi]]
                )
            return xts[mi]

        # Main fused matmul: out(n, m) += sum_k wac[k, n].T @ facts_t[k, m]
        def one_pass():
            for mi in range(len(widths)):
                xt = get_xt(mi)
                # mwidth: timing-only variant that streams a narrower slice
                # through the PE while keeping instruction count, DMA sizes,
                # and structure identical (isolates per-row PE time).
                mw = min(mwidth, widths[mi]) if mwidth else widths[mi]
                for n in range(NN):
                    ps = pspool.tile(
                        [128, widths[mi]], F32, tag="ps", name="ps",
                        padded_shape=[128, wmax],
                    )
                    for k in range(KT):
                        nc.tensor.matmul(
                            ps[:, 0:mw],
                            wt[:, k, n * 128 : (n + 1) * 128],
                            xt[:, k, 0:mw],
                            start=(k == 0),
                            stop=(k == KT - 1),
                        )
                    if copy_out:
                        ot = opool.tile(
                            [128, widths[mi]], BF16, tag="ot", name="ot",
                            padded_shape=[128, wmax],
                        )
                        nc.vector.tensor_copy(ot[:], ps[:])
                        if dma_out:
                            nc.sync.dma_start(
                                pc_t[
                                    n * 128 : (n + 1) * 128,
                                    starts[mi] : starts[mi] + widths[mi],
                                ],
                                ot[:],
                            )
            if redma:
                xts.clear()

        if hw_loop:
            # Same instruction stream for any rep count (loop bound is an
            # immediate) — the timing-only mode that decouples executable
            # size from executed passes.
            with tc.For_i(0, reps):
                one_pass()
        else:
            for _ in range(reps):
                one_pass()
    return _legalize_sync(nc)


def _shard_inputs(facts, Wc):
    import ml_dtypes

    bf16 = ml_dtypes.bfloat16
    fx = np.ascontiguousarray(facts, dtype=np.float32).reshape(BL, V)
    Wc = np.asarray(Wc, dtype=np.float32)
    wac_full = np.concatenate([Wc[0], Wc[1], Wc[2]], axis=1)  # (V, 768)

    in_maps = []
    for c in range(NCORES):
        sl = slice(c * VSH, (c + 1) * VSH)
        ft = np.zeros((VPAD, BL), bf16)
        ft[:VSH] = fx[:, sl].T.astype(bf16)
        ws = np.zeros((VPAD, NF), bf16)
        ws[:VSH] = wac_full[sl].astype(bf16)
        in_maps.append({"facts_t": ft, "wac": ws})
    return in_maps


def _wait_for_devices(min_wait_attempts=10):
    """The axon terminal occasionally reports a transient bad topology
    ("terminal has 1 core"); poll until all 8 NeuronCores are visible."""
    import time as _time

    import jax

    for attempt in range(min_wait_attempts):
        try:
            if len(jax.devices()) >= NCORES:
                return
        except Exception:  # noqa: BLE001 - backend init failure is retryable
            try:
                jax.clear_backends()
            except Exception:  # noqa: BLE001
                pass
        _time.sleep(15.0)
    # fall through: let the run itself raise a descriptive error


def _run_with_retries(nc, in_maps, attempts=4):
    """run_bass_kernel_spmd with retries: the axon terminal occasionally
    reports transient failures (device wedged / NRT_EXEC_UNIT_UNRECOVERABLE /
    temporary topology glitches) that succeed on re-dispatch."""
    import time as _time

    last_exc = None
    for attempt in range(attempts):
        try:
            return run_bass_kernel_spmd(nc, in_maps, list(range(NCORES)))
        except Exception as e:  # noqa: BLE001 - retry any runtime failure
            last_exc = e
            if attempt < attempts - 1:
                _time.sleep(10.0 * (attempt + 1))
                _wait_for_devices(min_wait_attempts=4)
    raise last_exc


def kernel(facts, question, Wq, Wa, Wc, Ww, bw):
    global _nc_cache, _last_result
    _wait_for_devices(min_wait_attempts=8)
    facts = np.asarray(facts, dtype=np.float32)
    in_maps = _shard_inputs(facts, Wc)
    if _nc_cache is None:
        _nc_cache = _build()
    _last_result = _run_with_retries(_nc_cache, in_maps)
    res = _last_result.results

    # Unshard: sum the 8 partial products of the vocab-sharded matmul.
    c_t = res[0]["pc_t"].astype(np.float32)
    for r in res[1:]:
        c_t += r["pc_t"].astype(np.float32)
    # c_t is (768, 3200): rows = [C0 | C1 | C2] over d, cols = (b, l)

    # Sequential hop recurrence with the reassociated match (A_h is never
    # materialized): match[b,l] = facts[b,l,:] . (Wa[h]^T u[b]).
    question = np.asarray(question, dtype=np.float32)
    Wq = np.asarray(Wq, dtype=np.float32)
    Wa = np.asarray(Wa, dtype=np.float32)
    Ww = np.asarray(Ww, dtype=np.float32)
    bw = np.asarray(bw, dtype=np.float32)

    u = question.sum(axis=1) @ Wq                       # (B, D)
    for h in range(HOPS):
        wt = u @ Wa[h].T                                # (B, V)
        match = np.einsum("blv,bv->bl", facts, wt, optimize=True)
        mm = match - match.max(axis=-1, keepdims=True)
        e = np.exp(mm)
        p = e / e.sum(axis=-1, keepdims=True)           # (B, L)
        C = c_t[h * D : (h + 1) * D].reshape(D, B, L)   # (D, B, L)
        att = np.einsum("bl,dbl->bd", p, C, optimize=True)
        z = (u + att) @ Ww[h] + bw[h]
        if h == HOPS - 1:
            zz = z - z.max(axis=-1, keepdims=True)
            ez = np.exp(zz)
            u = ez / ez.sum(axis=-1, keepdims=True)
        else:
            u = np.maximum(z, 0.0)
    return np.ascontiguousarray(u, dtype=np.float32)


# revision 21
# speedup vs baseline: 1.2146x; 1.2146x over previous
"""MemNN (end-to-end memory network) Trainium2 kernel.

Algorithmic restructuring vs the naive reference: the reference materializes
A_h = facts @ Wa[h] (B,L,D) only to contract it with u into match = A_h . u.
Reassociating, match[b,l] = facts[b,l,:] . (Wa[h]^T u[b]) — the (B,L,D)
A-tensor never needs to exist, killing half of the 98.3 GFLOP.  What remains
u-independent (and therefore device-batchable in one launch) is the output
embedding C_h = facts @ Wc[h]: the three hops fuse into a single
(3200, 10000) @ (10000, 768) matmul = 49.2 GFLOP, which this kernel runs in
bf16 across 8 cores.  The sequential hop recurrence (match via the
reassociated form, softmax, att = p.C, Dense update) is ~1.3 GFLOP and runs
on the host, where `facts` is already resident — a device round-trip per hop
would cost ~100 ms of RPC latency for ~15 us of device math.

Sharding: vocab (contraction) dim split 8 ways -> each core reads only its
1/8 slice of facts/Wc (~10 MB/core), computes a partial product at full PE
rate (bf16), and writes it to DRAM.  The host unshards by summing the 8
partials in fp32.

Precision: bf16 inputs with fp32 PSUM accumulation; empirically the
end-to-end relative error is ~6e-3 against the fp32 reference (gate 2e-2).
"""

import os

os.environ.setdefault("MYCRO_LOCAL_CACHE", "1")

import numpy as np

import concourse.bass as bass
import concourse.mybir as mybir
import concourse.tile as tile
from concourse.bass_utils import run_bass_kernel_spmd

HOPS, B, L, V, D = 3, 64, 50, 10000, 256
NCORES = 8
BL = B * L                # 3200 moving rows
NF = HOPS * D             # 768 fused output cols: [Wc0|Wc1|Wc2]
VSH = V // NCORES         # 1250 vocab rows per core
KT = 10                   # contraction tiles of 128 per core
VPAD = KT * 128           # 1280 (zero-padded)
MCH = 400                 # moving-col chunk: 8 even chunks; lowest model
                          # pass+const total (512-chunking saves ~1us of pass
                          # but its 128-wide tail adds ~6us of epilogue drain)
NN = NF // 128            # 6 stationary W tiles
BF16 = mybir.dt.bfloat16
F32 = mybir.dt.float32

_nc_cache = None
_last_result = None       # BassKernelResults of the most recent run (for profiling)


def _legalize_sync(nc):
    """Split multi-wait sync_info into standalone single-wait EventSemaphores.

    The walrus build in this environment enforces the raw-bass contract of at
    most ONE SyncWait per instruction ("Too many sync wait commands" in
    setupSyncWait otherwise), while Tile attaches every needed wait to the
    consuming instruction.  Hoisting all-but-one wait onto preceding
    InstEventSemaphore instructions on the same engine queue is semantically
    identical: engine queues are in-order, so a preceding wait blocks the
    queue exactly like an attached wait.  Updates are left untouched (they
    fire at completion and cannot be hoisted).
    """
    for func in nc.m.functions:
        for block in func.blocks:
            insts = list(block.instructions)
            out = []
            n = 0
            for inst in insts:
                si = inst.sync_info
                if si is not None and len(si.on_wait) > 1:
                    waits = list(si.on_wait)
                    for w in waits[:-1]:
                        ev = mybir.InstEventSemaphore(
                            name=f"{inst.name}-hoistw{n}", ins=[], outs=[]
                        )
                        n += 1
                        ev.engine = inst.engine
                        ev.sync_info = mybir.SyncInfo(on_wait=[w], on_update=[])
                        nc.register_instruction(ev)
                        out.append(ev)
                    inst.sync_info = mybir.SyncInfo(
                        on_wait=[waits[-1]], on_update=list(si.on_update)
                    )
                out.append(inst)
            if len(out) != len(insts):
                block.instructions = out
    return nc


# Moving-dim chunking of the 3200 BL columns.
def _chunk_widths(mch):
    if mch == 512:
        return [512] * 6 + [128]      # 3200 = 6*512 + 128
    assert BL % mch == 0, mch
    return [mch] * (BL // mch)


_WIDTHS = _chunk_widths(MCH)
_STARTS = [sum(_WIDTHS[:i]) for i in range(len(_WIDTHS))]
assert sum(_WIDTHS) == BL


def _build(reps=1, redma=True, dma_out=True, copy_out=True, xbufs=3, mch=None,
           mwidth=None, hw_loop=False):
    """Build the SPMD device program.

    reps>1 repeats the main loop body (same data, same output addresses) —
    used only by the benchmark harness to measure device time differentially
    (per-call dispatch noise over the axon tunnel is ~ms, device time is
    ~200 us, so wall-clocking one launch cannot resolve it).

    redma/dma_out/copy_out are micro-benchmark toggles (timing variants
    only): re-DMA facts each rep / write pc_t / evacuate PSUM.
    """
    widths = _chunk_widths(mch) if mch else _WIDTHS
    starts = [sum(widths[:i]) for i in range(len(widths))]
    nc = bass.Bass(trn_type="TRN2")
    facts_t = nc.dram_tensor("facts_t", [VPAD, BL], BF16, kind="ExternalInput")
    wac = nc.dram_tensor("wac", [VPAD, NF], BF16, kind="ExternalInput")
    pc_t = nc.dram_tensor("pc_t", [NF, BL], BF16, kind="ExternalOutput")

    fr = facts_t.rearrange("(k p) n -> p k n", p=128)
    wr = wac.rearrange("(k p) n -> p k n", p=128)
    wmax = max(widths)

    with (
        tile.TileContext(nc) as tc,
        tc.tile_pool(name="wpool", bufs=1) as wpool,
        tc.tile_pool(name="xpool", bufs=xbufs) as xpool,
        tc.tile_pool(name="opool", bufs=4) as opool,
        tc.tile_pool(name="pspool", bufs=6, space="PSUM") as pspool,
    ):
        # Prologue DMA order: first wac n-slice 0 + first facts chunk (the
        # first matmul group's deps), then the rest of wac.
        wt = wpool.tile([128, KT, NF], BF16)
        nc.sync.dma_start(wt[:, :, 0:128], wr[:, :, 0:128])
        xts = {}
        xts[0] = xpool.tile(
            [128, KT, widths[0]], BF16, tag="xt", name="xt",
            padded_shape=[128, KT, wmax],
        )
        nc.sync.dma_start(xts[0][:], fr[:, :, 0 : widths[0]])
        for off in range(128, NF, 512):
            end = min(off + 512, NF)
            nc.sync.dma_start(wt[:, :, off:end], wr[:, :, off:end])

        def get_xt(mi):
            if mi not in xts:
                xts[mi] = xpool.tile(
                    [128, KT, widths[mi]], BF16, tag="xt", name="xt",
                    padded_shape=[128, KT, wmax],
                )
                nc.sync.dma_start(
                    xts[mi][:], fr[:, :, starts[mi] : starts[mi] + widths[mi]]
                )
            return xts[mi]

        # Main fused matmul: out(n, m) += sum_k wac[k, n].T @ facts_t[k, m]
        def one_pass():
            for mi in range(len(widths)):
                xt = get_xt(mi)
                # mwidth: timing-only variant that streams a narrower slice
                # through the PE while keeping instruction count, DMA sizes,
                # and structure identical (isolates per-row PE time).
                mw = min(mwidth, widths[mi]) if mwidth else widths[mi]
                for n in range(NN):
                    ps = pspool.tile(
                        [128, widths[mi]], F32, tag="ps", name="ps",
                        padded_shape=[128, wmax],
                    )
                    for k in range(KT):
                        nc.tensor.matmul(
                            ps[:, 0:mw],
                            wt[:, k, n * 128 : (n + 1) * 128],
                            xt[:, k, 0:mw],
                            start=(k == 0),
                            stop=(k == KT - 1),
                        )
                    if copy_out:
                        ot = opool.tile(
                            [128, widths[mi]], BF16, tag="ot", name="ot",
                            padded_shape=[128, wmax],
                        )
                        nc.vector.tensor_copy(ot[:], ps[:])
                        if dma_out:
                            nc.sync.dma_start(
                                pc_t[
                                    n * 128 : (n + 1) * 128,
                                    starts[mi] : starts[mi] + widths[mi],
                                ],
                                ot[:],
                            )
            if redma:
                xts.clear()

        if hw_loop:
            # Same instruction stream for any rep count (loop bound is an
            # immediate) — the timing-only mode that decouples executable
            # size from executed passes.
            with tc.For_i(0, reps):
                one_pass()
        else:
            for _ in range(reps):
                one_pass()
    return _legalize_sync(nc)


def _shard_inputs(facts, Wc):
    import ml_dtypes

    bf16 = ml_dtypes.bfloat16
    fx = np.ascontiguousarray(facts, dtype=np.float32).reshape(BL, V)
    Wc = np.asarray(Wc, dtype=np.float32)
    wac_full = np.concatenate([Wc[0], Wc[1], Wc[2]], axis=1)  # (V, 768)

    in_maps = []
    for c in range(NCORES):
        sl = slice(c * VSH, (c + 1) * VSH)
        ft = np.zeros((VPAD, BL), bf16)
        ft[:VSH] = fx[:, sl].T.astype(bf16)
        ws = np.zeros((VPAD, NF), bf16)
        ws[:VSH] = wac_full[sl].astype(bf16)
        in_maps.append({"facts_t": ft, "wac": ws})
    return in_maps


def _wait_for_devices(min_wait_attempts=10):
    """The axon terminal occasionally reports a transient bad topology
    ("terminal has 1 core"); poll until all 8 NeuronCores are visible."""
    import time as _time

    import jax

    for attempt in range(min_wait_attempts):
        try:
            if len(jax.devices()) >= NCORES:
                return
        except Exception:  # noqa: BLE001 - backend init failure is retryable
            try:
                jax.clear_backends()
            except Exception:  # noqa: BLE001
                pass
        _time.sleep(15.0)
    # fall through: let the run itself raise a descriptive error


def _run_with_retries(nc, in_maps, attempts=4):
    """run_bass_kernel_spmd with retries: the axon terminal occasionally
    reports transient failures (device wedged / NRT_EXEC_UNIT_UNRECOVERABLE /
    temporary topology glitches) that succeed on re-dispatch."""
    import time as _time

    last_exc = None
    for attempt in range(attempts):
        try:
            return run_bass_kernel_spmd(nc, in_maps, list(range(NCORES)))
        except Exception as e:  # noqa: BLE001 - retry any runtime failure
            last_exc = e
            if attempt < attempts - 1:
                _time.sleep(10.0 * (attempt + 1))
                _wait_for_devices(min_wait_attempts=4)
    raise last_exc


def kernel(facts, question, Wq, Wa, Wc, Ww, bw):
    global _nc_cache, _last_result
    _wait_for_devices(min_wait_attempts=8)
    facts = np.asarray(facts, dtype=np.float32)
    in_maps = _shard_inputs(facts, Wc)
    if _nc_cache is None:
        _nc_cache = _build()
    _last_result = _run_with_retries(_nc_cache, in_maps)
    res = _last_result.results

    # Unshard: sum the 8 partial products of the vocab-sharded matmul.
    c_t = res[0]["pc_t"].astype(np.float32)
    for r in res[1:]:
        c_t += r["pc_t"].astype(np.float32)
    # c_t is (768, 3200): rows = [C0 | C1 | C2] over d, cols = (b, l)

    # Sequential hop recurrence with the reassociated match (A_h is never
    # materialized): match[b,l] = facts[b,l,:] . (Wa[h]^T u[b]).
    question = np.asarray(question, dtype=np.float32)
    Wq = np.asarray(Wq, dtype=np.float32)
    Wa = np.asarray(Wa, dtype=np.float32)
    Ww = np.asarray(Ww, dtype=np.float32)
    bw = np.asarray(bw, dtype=np.float32)

    u = question.sum(axis=1) @ Wq                       # (B, D)
    for h in range(HOPS):
        wt = u @ Wa[h].T                                # (B, V)
        match = np.einsum("blv,bv->bl", facts, wt, optimize=True)
        mm = match - match.max(axis=-1, keepdims=True)
        e = np.exp(mm)
        p = e / e.sum(axis=-1, keepdims=True)           # (B, L)
        C = c_t[h * D : (h + 1) * D].reshape(D, B, L)   # (D, B, L)
        att = np.einsum("bl,dbl->bd", p, C, optimize=True)
        z = (u + att) @ Ww[h] + bw[h]
        if h == HOPS - 1:
            zz = z - z.max(axis=-1, keepdims=True)
            ez = np.exp(zz)
            u = ez / ez.sum(axis=-1, keepdims=True)
        else:
            u = np.maximum(z, 0.0)
    return np.ascontiguousarray(u, dtype=np.float32)
